# revision 8
# baseline (speedup 1.0000x reference)
"""Trainium2 Bass kernel for nn_NeuralGraphHidden (GNN message passing).

Structure: edges ~ randint(-1, 128) makes ~95.5% of atoms degree 6, whose
outputs are exactly zero (the reference's degree mask covers 0..5 only).  Of
the ~1440 "active" atoms, ~99% are degree 5.  The device handles ONLY the
degree-5 atoms (balanced across the 8 cores, NA~184/core); the handful of
degree<5 atoms are computed exactly on the host in numpy (microseconds).

Device pipeline per core (slot 5 is the padding slot: its neighbour term is
zero, so its bond-only message MLP m1_5 is evaluated on the host and shipped
in as `m15`; the 5 real edges are packed into slots 0-4):

  pm_g   = W0a.T @ nbrT_g + W0b.T @ bondT_g        (fp8 weights + data)
  m0_g   = elu(pm_g)                               (ACT exp + DVE fuse)
  pm2_g  = W1.T @ m0_g                             (bf16)
  m1_g   = elu(pm2_g)
  pi     = lo5.T @ nact + hi5.T @ m15 + sum_j hi5.T @ m1_j   (PSUM accum)
  h0     = elu(pi)
  out    = elu(iw15.T @ h0)                        -> bf16 DMA out

elu(x) = relu(x) + min(exp(x), 1) - 1: exp on the ACT engine (bf16 out), the
combine as one fused custom-DVE op.  Slot groups (0,1),(2,3),(4); the two
full-width groups are processed as single merged two-bank ACT/DVE ops to
amortize per-op overheads.  Inputs ride 3 DMA queue families (SP-HWDGE,
ACT-HWDGE, SWDGE) in need-order; an ACT-table prewarm and a PE clock-ramp
matmul burst run during the initial DMA wait.
"""

import sys

if "/opt/trn_rl_repo" not in sys.path:
    sys.path.insert(0, "/opt/trn_rl_repo")

import numpy as np
import ml_dtypes

import concourse.bass as bass
import concourse.bacc as bacc
import concourse.mybir as mybir
import concourse.tile as tile
from concourse import bass_utils

import concourse.dve_ops as dve_ops
from concourse.dve_spec import (Spec, Src0, Src1, C0, C1, Zero, maxx, minn,
                                lower)
from concourse.dve_uop import DveOpSpec


def _make_elu_op():
    """out = relu(in0) + min(in1, c0) + c1  -- with c0=1, c1=-1 and
    in1=exp(in0) this is exactly elu(in0)."""
    name = "ELU_FUSED_ANT"
    for op in dve_ops.OPS:
        if op.name == name:
            return op
    spec = Spec(
        body=maxx(Src0, Zero) + minn(Src1, C0) + C1,
        reference=lambda in0, in1, c0, c1, c2: (
            np.maximum(in0.astype(np.float32).reshape(in0.shape[0], -1), 0)
            + np.minimum(in1.astype(np.float32).reshape(in1.shape[0], -1), c0)
            + c1),
    )
    idx = dve_ops._CUSTOM_DVE_ROW_BASE + len(dve_ops.OPS)
    shas = {}
    for ver in ("v3", "v4"):
        compiled = DveOpSpec(name=name, opcode=idx, uops=lower(spec, ver=ver),
                             rd1_en=True)
        shas[ver] = compiled.sha(ver)
    op = dve_ops.DveOp(name, spec, subdim=False, uops_sha=shas)
    dve_ops.OPS.append(op)
    dve_ops.CUSTOM_DVE_SPECS[name] = spec
    dve_ops._SUB_OPCODE_FOR_NAME[name] = idx
    return op


ELU_OP = _make_elu_op()

BF16 = ml_dtypes.bfloat16
FP8 = ml_dtypes.float8_e4m3fn
F32 = mybir.dt.float32
BF = mybir.dt.bfloat16
F8 = mybir.dt.float8e4
AF = mybir.ActivationFunctionType
ALU = mybir.AluOpType

B, M, D = 256, 128, 6
FA, FB, MSG, CONV = 128, 32, 128, 128
NCORES = 8

WARMUP_MMS = 5       # PE clock-ramp burst during the initial DMA wait


def _roundup(x, m):
    return (x + m - 1) // m * m


# --------------------------------------------------------------------------
# device program
# --------------------------------------------------------------------------

def build_program(NA, warmup=WARMUP_MMS):
    """SPMD program: NA degree-5 atom slots per core (multiple of 8)."""
    nc = bacc.Bacc("TRN2", target_bir_lowering=False, debug=False,
                   enable_asserts=False, num_devices=NCORES)

    # aw:   bf16 [128, 256]:      w0a | w1                  (Sync HWDGE)
    # b8:   bf16 [32, 128+5*NA]:  w0b | bopT slots 0..4     (Scalar HWDGE)
    # nap8: fp8  [128, 5*NA]:     napT slots 0..4           (SWDGE, 2 chunks)
    # wb:   bf16 [128, 384+2*NA]: lo5 | hi5 | iw15 | nact | m15  (SWDGE)
    aw_d = nc.dram_tensor("aw", [128, 256], BF, kind="ExternalInput").ap()
    b8_d = nc.dram_tensor("b8", [32, 128 + 5 * NA], BF,
                          kind="ExternalInput").ap()
    nap8_d = nc.dram_tensor("nap8", [128, 5 * NA], F8,
                            kind="ExternalInput").ap()
    wb_d = nc.dram_tensor("wb", [128, 384 + 2 * NA], BF,
                          kind="ExternalInput").ap()
    outp = nc.dram_tensor("outp", [128, NA], BF, kind="ExternalOutput")
    outp_ap = outp.ap()

    H = NA - 16  # first (large) output chunk; tiny last chunk for the tail

    with tile.TileContext(nc) as tc:
        with (
            tc.tile_pool(name="w", bufs=1) as wp,
            tc.tile_pool(name="work", bufs=2) as work,
            tc.tile_pool(name="ps", bufs=1, space=bass.MemorySpace.PSUM) as ps,
            tc.tile_pool(name="pio", bufs=2, space=bass.MemorySpace.PSUM) as pio,
        ):
            aw = wp.tile([128, 256], BF, tag="aw")
            b8 = wp.tile([32, 128 + 5 * NA], BF, tag="b8")
            nap8 = wp.tile([128, 5 * NA], F8, tag="nap8")
            wb = wp.tile([128, 384 + 2 * NA], BF, tag="wb")

            # ---- input DMAs (need-order, 3 queue families) ---------------
            nc.sync.dma_start(aw[:], aw_d[:])                      # w0a + w1
            nc.scalar.dma_start(b8[:], b8_d[:])                    # w0b + bop
            nc.gpsimd.dma_start(nap8[:, 0:2 * NA], nap8_d[:, 0:2 * NA])
            nc.gpsimd.dma_start(nap8[:, 2 * NA:], nap8_d[:, 2 * NA:])
            nc.gpsimd.dma_start(wb[:], wb_d[:])                    # inner etc.

            w0a = aw[:, 0:128]
            w1 = aw[:, 128:256]
            w0b = b8[:, 0:128]
            lo5 = wb[:, 0:128]
            hi5 = wb[:, 128:256]
            iw15 = wb[:, 256:384]
            nact = wb[:, 384:384 + NA]
            m15 = wb[:, 384 + NA:384 + 2 * NA]

            def nap(s0, s1):  # nbr slots [s0, s1)
                return nap8[:, s0 * NA:s1 * NA]

            def bop(s0, s1):
                return b8[:, 128 + s0 * NA:128 + s1 * NA]

            # ---- PE clock-ramp burst + ACT exp-table prewarm -------------
            wz = wp.tile([128, 512], BF, tag="wz")
            nc.vector.memset(wz[:], 0.0)
            escr = wp.tile([128, 1], F32, tag="escr")
            nc.scalar.activation(escr[:], wz[:, 0:1], AF.Exp)
            if warmup:
                pw = pio.tile([128, 512], F32, tag="pio")
                for _ in range(warmup):
                    nc.tensor.matmul(pw[:], wz[:, 0:128], wz[:],
                                     start=True, stop=True)

            # ---- msg layer 0: groups (0,1),(2,3),(4) ---------------------
            # bond matmuls first (b8 lands earliest; keeps PE busy/warm),
            # then the neighbour matmuls as the fp8 nap chunks arrive.
            pmA = ps.tile([128, 2, 512], F32, tag="pmA")
            pmB = ps.tile([128, 512], F32, tag="pmB")
            for g in range(2):
                nc.tensor.matmul(pmA[:, g, 0:2 * NA], w0b, bop(2 * g, 2 * g + 2),
                                 start=True, stop=False)
            nc.tensor.matmul(pmB[:, 0:NA], w0b, bop(4, 5), start=True, stop=False)
            for g in range(2):
                nc.tensor.matmul(pmA[:, g, 0:2 * NA], w0a, nap(2 * g, 2 * g + 2),
                                 start=False, stop=True)
            nc.tensor.matmul(pmB[:, 0:NA], w0a, nap(4, 5), start=False, stop=True)

            # elu: exp on ACT, fused combine on DVE (GPSIMD can't read PSUM)
            def elu_tile(pv, out_ap, cols, tag):
                """pv: PSUM f32 AP; out_ap: SBUF bf16 dest; cols: elems."""
                e = work.tile([128, cols], BF, tag=tag)
                nc.scalar.activation(e[:], pv, AF.Exp)
                nc.vector._custom_dve(ELU_OP, out=out_ap, in0=pv,
                                      in1=e[:], s0=1.0, s1=-1.0)

            m0A = wp.tile([128, 2, 2 * NA], BF, tag="m0A")
            m0B = wp.tile([128, NA], BF, tag="m0B")
            for g in range(2):
                elu_tile(pmA[:, g, 0:2 * NA], m0A[:, g, :], 2 * NA, f"eA{g}")
            elu_tile(pmB[:, 0:NA], m0B[:], NA, "eB")

            # ---- msg layer 1 --------------------------------------------
            pm2A = ps.tile([128, 2, 512], F32, tag="pm2A")
            pm2B = ps.tile([128, 512], F32, tag="pm2B")
            for g in range(2):
                nc.tensor.matmul(pm2A[:, g, 0:2 * NA], w1, m0A[:, g, :],
                                 start=True, stop=True)
            nc.tensor.matmul(pm2B[:, 0:NA], w1, m0B[:], start=True, stop=True)

            m1A = wp.tile([128, 2, 2 * NA], BF, tag="m1A")
            m1B = wp.tile([128, NA], BF, tag="m1B")
            for g in range(2):
                elu_tile(pm2A[:, g, 0:2 * NA], m1A[:, g, :], 2 * NA, f"e2A{g}")
            elu_tile(pm2B[:, 0:NA], m1B[:], NA, "e2B")

            # ---- inner layer 0 (degree-5 weights, PSUM accumulate) ------
            pi = pio.tile([128, 512], F32, tag="pio")
            nc.tensor.matmul(pi[:, 0:NA], lo5, nact, start=True, stop=False)
            nc.tensor.matmul(pi[:, 0:NA], hi5, m15, start=False, stop=False)
            for j in range(4):
                nc.tensor.matmul(pi[:, 0:NA], hi5,
                                 m1A[:, j // 2, (j % 2) * NA:(j % 2 + 1) * NA],
                                 start=False, stop=False)
            nc.tensor.matmul(pi[:, 0:NA], hi5, m1B[:], start=False, stop=True)
            h0 = wp.tile([128, NA], BF, tag="h0")
            elu_tile(pi[:, 0:NA], h0[:], NA, "eh")

            # ---- inner layer 1 + output (uneven chunks, two DMA queues) --
            obuf = wp.tile([128, NA], BF, tag="obuf")
            po_a = pio.tile([128, 512], F32, tag="pio")
            nc.tensor.matmul(po_a[:, 0:H], iw15, h0[:, 0:H],
                             start=True, stop=True)
            # reuse g2's msg0 bank (its fuse reader is long done by now)
            po_b = ps.tile([128, 512], F32, tag="pmB")
            nc.tensor.matmul(po_b[:, 0:NA - H], iw15, h0[:, H:NA],
                             start=True, stop=True)
            elu_tile(po_a[:, 0:H], obuf[:, 0:H], H, "eo1")
            nc.sync.dma_start(outp_ap[:, 0:H], obuf[:, 0:H])
            elu_tile(po_b[:, 0:NA - H], obuf[:, H:NA], NA - H, "eo2")
            nc.scalar.dma_start(outp_ap[:, H:NA], obuf[:, H:NA])

    nc.compile()
    return nc


_CACHE = {}


# --------------------------------------------------------------------------
# host side
# --------------------------------------------------------------------------

def _elu(x):
    return np.where(x > 0, x, np.expm1(np.minimum(x, 0.0)))


def _host_fallback(af, bf, ef, deg, ids, msg_w0, msg_w1, inner_w0, inner_w1):
    """Exact f32 reference for the (few) active atoms with degree < 5.
    af: (N,FA) atoms flat; bf: (N,D,FB); ef: (N,D); ids: flat atom indices."""
    if len(ids) == 0:
        return np.zeros((0, CONV), np.float32)
    mol = ids // M
    e = ef[ids]                                   # (n, D)
    nbr = np.where(e[..., None] >= 0,
                   af[(mol[:, None] * M + np.maximum(e, 0)).ravel()]
                   .reshape(len(ids), D, FA),
                   0.0)
    msg_in = np.concatenate([nbr, bf[ids]], axis=-1)        # (n, D, FA+FB)
    msg = _elu(msg_in @ msg_w0)
    msg = _elu(msg @ msg_w1)
    summed = msg.sum(axis=1)                                # (n, MSG)
    s2 = np.concatenate([summed, af[ids]], axis=-1)         # (n, MSG+FA)
    dg = deg[ids]
    h = _elu(np.einsum('nf,nfc->nc', s2, inner_w0[dg]))
    h = _elu(np.einsum('nc,nce->ne', h, inner_w1[dg]))
    return h.astype(np.float32)


def _prep_core(af, bf, ef, ids, NA, msg_w0, msg_w1):
    """Stage one core's deg-5 atoms (flat ids into af/bf/ef).
    Returns (napf [128,5,NA] f32, bopf [32,5,NA] f32, nact [128,NA] f32,
    m15 [128,NA] f32 — the host-computed padding-slot message)."""
    n = len(ids)
    mol = ids // M
    e = ef[ids]                                   # (n, 6), exactly one -1
    real = e >= 0                                 # (n, 6) 5 True per row
    # pack real edges into slots 0-4; the pad slot's bond goes to the host
    order = np.argsort(~real, axis=1, kind="stable")   # real first
    e_p = np.take_along_axis(e, order, axis=1)         # (n,6) col5 = -1
    b_p = np.take_along_axis(bf[ids], order[..., None], axis=1)  # (n,6,FB)

    src = af[(mol[:, None] * M + e_p[:, :5]).ravel()].reshape(n, 5, FA)
    napf = np.zeros((128, 5, NA), np.float32)
    napf[:, :, :n] = src.transpose(2, 1, 0)
    bopf = np.zeros((32, 5, NA), np.float32)
    bopf[:, :, :n] = b_p[:, :5].transpose(2, 1, 0)
    nact = np.zeros((128, NA), np.float32)
    nact[:, :n] = af[ids].T
    # padding-slot bond message: nbr contribution is zero
    m15v = _elu(_elu(b_p[:, 5] @ msg_w0[FA:]) @ msg_w1)   # (n, MSG)
    m15 = np.zeros((128, NA), np.float32)
    m15[:, :n] = m15v.T
    return napf, bopf, nact, m15


def prepare_in_maps(atoms, bonds, edges, msg_w0, msg_w1, inner_w0, inner_w1):
    """Shared by kernel() and test.py: returns (NA, per_core, rest, in_maps)."""
    af = atoms.reshape(B * M, FA)
    bf = bonds.reshape(B * M, D, FB)
    ef = edges.reshape(B * M, D)
    deg = (ef != -1).sum(-1)

    d5 = np.nonzero(deg == 5)[0]
    rest = np.nonzero(deg < 5)[0]

    per_core = [d5[c::NCORES] for c in range(NCORES)]
    NA = max(16, _roundup(max(len(p) for p in per_core), 8))

    awx = np.zeros((128, 256), np.float32)
    awx[:, 0:128] = msg_w0[:FA]
    awx[:, 128:256] = msg_w1
    aw = awx.astype(BF16)
    w0b16 = msg_w0[FA:].astype(BF16)                   # [32,128]
    wbase = np.zeros((128, 384), np.float32)
    wbase[:, 0:128] = inner_w0[5, 128:, :]   # lo5: atom-feature part
    wbase[:, 128:256] = inner_w0[5, :128, :]  # hi5: summed-message part
    wbase[:, 256:384] = inner_w1[5]

    in_maps = []
    for c in range(NCORES):
        ids = per_core[c]
        napf, bopf, nact, m15 = _prep_core(af, bf, ef, ids, NA,
                                           msg_w0, msg_w1)
        nap8 = napf.reshape(128, 5 * NA).astype(FP8)
        b8 = np.zeros((32, 128 + 5 * NA), BF16)
        b8[:, 0:128] = w0b16
        b8[:, 128:] = bopf.reshape(32, 5 * NA).astype(BF16)
        wbx = np.zeros((128, 384 + 2 * NA), np.float32)
        wbx[:, 0:384] = wbase
        wbx[:, 384:384 + NA] = nact
        wbx[:, 384 + NA:] = m15
        in_maps.append({"aw": aw, "b8": b8, "nap8": nap8,
                        "wb": wbx.astype(BF16)})
    return NA, per_core, rest, in_maps


def kernel(atoms, bonds, edges, msg_w0, msg_w1, inner_w0, inner_w1):
    atoms = np.asarray(atoms, np.float32)
    bonds = np.asarray(bonds, np.float32)
    edges = np.asarray(edges, np.int32)
    msg_w0 = np.asarray(msg_w0, np.float32)
    msg_w1 = np.asarray(msg_w1, np.float32)
    inner_w0 = np.asarray(inner_w0, np.float32)
    inner_w1 = np.asarray(inner_w1, np.float32)

    NA, per_core, rest, in_maps = prepare_in_maps(
        atoms, bonds, edges, msg_w0, msg_w1, inner_w0, inner_w1)

    if NA not in _CACHE:
        _CACHE[NA] = build_program(NA)
    nc = _CACHE[NA]

    res = bass_utils.run_bass_kernel_spmd(
        nc, in_maps, core_ids=list(range(NCORES)))

    af = atoms.reshape(B * M, FA)
    bf = bonds.reshape(B * M, D, FB)
    ef = edges.reshape(B * M, D)
    deg = (ef != -1).sum(-1)

    out = np.zeros((B * M, CONV), np.float32)
    for c in range(NCORES):
        ids = per_core[c]
        o = np.asarray(res.results[c]["outp"]).astype(np.float32)  # (128, NA)
        out[ids] = o[:, :len(ids)].T
    out[rest] = _host_fallback(af, bf, ef, deg, rest,
                               msg_w0, msg_w1, inner_w0, inner_w1)
    return out.reshape(B, M, CONV)


# revision 10
# speedup vs baseline: 1.0824x; 1.0824x over previous
"""Trainium2 Bass kernel for nn_NeuralGraphHidden (GNN message passing).

Structure: edges ~ randint(-1, 128) makes ~95.5% of atoms degree 6, whose
outputs are exactly zero (the reference's degree mask covers 0..5 only).  Of
the ~1440 "active" atoms, ~99% are degree 5.  The device handles ONLY the
degree-5 atoms (balanced across the 8 cores, NA~184/core); the handful of
degree<5 atoms are computed exactly on the host in numpy (microseconds).

Device pipeline per core (slot 5 is the padding slot: its neighbour term is
zero, so its bond-only message MLP m1_5 is evaluated on the host and shipped
in as `m15`; the 5 real edges are packed into slots 0-4):

  pm_g   = W0a.T @ nbrT_g + W0b.T @ bondT_g        (fp8 weights + data)
  m0_g   = elu(pm_g)                               (ACT exp + DVE fuse)
  pm2_g  = W1.T @ m0_g                             (bf16)
  m1_g   = elu(pm2_g)
  pi     = lo5.T @ nact + hi5.T @ m15 + sum_j hi5.T @ m1_j   (PSUM accum)
  h0     = elu(pi)
  out    = elu(iw15.T @ h0)                        -> bf16 DMA out

elu(x) = relu(x) + min(exp(x), 1) - 1: exp on the ACT engine (bf16 out), the
combine as one fused custom-DVE op.  Slot groups (0,1),(2,3),(4); the two
full-width groups are processed as single merged two-bank ACT/DVE ops to
amortize per-op overheads.  Inputs ride 3 DMA queue families (SP-HWDGE,
ACT-HWDGE, SWDGE) in need-order; an ACT-table prewarm and a PE clock-ramp
matmul burst run during the initial DMA wait.
"""

import sys

if "/opt/trn_rl_repo" not in sys.path:
    sys.path.insert(0, "/opt/trn_rl_repo")

import numpy as np
import ml_dtypes

import concourse.bass as bass
import concourse.bacc as bacc
import concourse.mybir as mybir
import concourse.tile as tile
from concourse import bass_utils

import concourse.dve_ops as dve_ops
from concourse.dve_spec import (Spec, Src0, Src1, C0, C1, Zero, maxx, minn,
                                lower)
from concourse.dve_uop import DveOpSpec


def _make_elu_op():
    """out = relu(in0) + min(in1, c0) + c1  -- with c0=1, c1=-1 and
    in1=exp(in0) this is exactly elu(in0)."""
    name = "ELU_FUSED_ANT"
    for op in dve_ops.OPS:
        if op.name == name:
            return op
    spec = Spec(
        body=maxx(Src0, Zero) + minn(Src1, C0) + C1,
        reference=lambda in0, in1, c0, c1, c2: (
            np.maximum(in0.astype(np.float32).reshape(in0.shape[0], -1), 0)
            + np.minimum(in1.astype(np.float32).reshape(in1.shape[0], -1), c0)
            + c1),
    )
    idx = dve_ops._CUSTOM_DVE_ROW_BASE + len(dve_ops.OPS)
    shas = {}
    for ver in ("v3", "v4"):
        compiled = DveOpSpec(name=name, opcode=idx, uops=lower(spec, ver=ver),
                             rd1_en=True)
        shas[ver] = compiled.sha(ver)
    op = dve_ops.DveOp(name, spec, subdim=False, uops_sha=shas)
    dve_ops.OPS.append(op)
    dve_ops.CUSTOM_DVE_SPECS[name] = spec
    dve_ops._SUB_OPCODE_FOR_NAME[name] = idx
    return op


ELU_OP = _make_elu_op()

BF16 = ml_dtypes.bfloat16
FP8 = ml_dtypes.float8_e4m3fn
F32 = mybir.dt.float32
BF = mybir.dt.bfloat16
F8 = mybir.dt.float8e4
AF = mybir.ActivationFunctionType
ALU = mybir.AluOpType

B, M, D = 256, 128, 6
FA, FB, MSG, CONV = 128, 32, 128, 128
NCORES = 8

WARMUP_MMS = 5       # PE clock-ramp burst during the initial DMA wait


def _roundup(x, m):
    return (x + m - 1) // m * m


# --------------------------------------------------------------------------
# device program
# --------------------------------------------------------------------------

def build_program(NA, warmup=WARMUP_MMS):
    """SPMD program: NA degree-5 atom slots per core (multiple of 8)."""
    nc = bacc.Bacc("TRN2", target_bir_lowering=False, debug=False,
                   enable_asserts=False, num_devices=NCORES)

    # aw:  bf16 [128, 256]:      w0a | w1                  (Sync HWDGE)
    # b8:  bf16 [32, 128+5*NA]:  w0b | bopT slots 0..4     (Scalar HWDGE, 2 chunks)
    # nap: bf16 [128, 5*NA]:     napT slots 0..4           (SWDGE, 2 chunks)
    # wb:  bf16 [128, 384+2*NA]: lo5 | hi5 | iw15 | nact | m15  (SWDGE)
    aw_d = nc.dram_tensor("aw", [128, 256], BF, kind="ExternalInput").ap()
    b8_d = nc.dram_tensor("b8", [32, 128 + 5 * NA], BF,
                          kind="ExternalInput").ap()
    nap_d = nc.dram_tensor("nap", [128, 5 * NA], BF,
                           kind="ExternalInput").ap()
    wb_d = nc.dram_tensor("wb", [128, 384 + 2 * NA], BF,
                          kind="ExternalInput").ap()
    outp = nc.dram_tensor("outp", [128, NA], BF, kind="ExternalOutput")
    outp_ap = outp.ap()

    H = NA - 16  # first (large) output chunk; tiny last chunk for the tail

    with tile.TileContext(nc) as tc:
        with (
            tc.tile_pool(name="w", bufs=1) as wp,
            tc.tile_pool(name="work", bufs=1) as work,
            tc.tile_pool(name="psM", bufs=3, space=bass.MemorySpace.PSUM) as psM,
            tc.tile_pool(name="psN", bufs=3, space=bass.MemorySpace.PSUM) as psN,
            tc.tile_pool(name="pio", bufs=2, space=bass.MemorySpace.PSUM) as pio,
        ):
            aw = wp.tile([128, 256], BF, tag="aw")
            b8 = wp.tile([32, 128 + 5 * NA], BF, tag="b8")
            napt = wp.tile([128, 5 * NA], BF, tag="napt")
            wb = wp.tile([128, 384 + 2 * NA], BF, tag="wb")

            # ---- input DMAs (need-order, 3 queue families) ---------------
            nc.sync.dma_start(aw[:], aw_d[:])                      # w0a + w1
            nc.scalar.dma_start(b8[:, 0:128 + 2 * NA],
                                b8_d[:, 0:128 + 2 * NA])           # w0b+bop01
            nc.scalar.dma_start(b8[:, 128 + 2 * NA:],
                                b8_d[:, 128 + 2 * NA:])            # bop234
            nc.gpsimd.dma_start(napt[:, 0:2 * NA], nap_d[:, 0:2 * NA])
            nc.gpsimd.dma_start(napt[:, 2 * NA:], nap_d[:, 2 * NA:])
            nc.gpsimd.dma_start(wb[:], wb_d[:])                    # inner etc.

            w0a = aw[:, 0:128]
            w1 = aw[:, 128:256]
            w0b = b8[:, 0:128]
            lo5 = wb[:, 0:128]
            hi5 = wb[:, 128:256]
            iw15 = wb[:, 256:384]
            nact = wb[:, 384:384 + NA]
            m15 = wb[:, 384 + NA:384 + 2 * NA]

            def nap(s0, s1):  # nbr slots [s0, s1)
                return napt[:, s0 * NA:s1 * NA]

            def bop(s0, s1):
                return b8[:, 128 + s0 * NA:128 + s1 * NA]

            # ---- PE clock-ramp burst + ACT exp-table prewarm -------------
            wz = wp.tile([128, 512], BF, tag="wz")
            nc.vector.memset(wz[:], 0.0)
            escr = wp.tile([128, 1], F32, tag="escr")
            nc.scalar.activation(escr[:], wz[:, 0:1], AF.Exp)
            if warmup:
                pw = pio.tile([128, 512], F32, tag="pio")
                for _ in range(warmup):
                    nc.tensor.matmul(pw[:], wz[:, 0:128], wz[:],
                                     start=True, stop=True)

            # ---- msg layer 0: groups (0,1),(2,3),(4) ---------------------
            # bond matmuls first (b8 lands earliest; keeps PE busy/warm),
            # then the neighbour matmuls as the nap chunks arrive.
            GW = [2 * NA, 2 * NA, NA]          # group widths
            GS = [(0, 2), (2, 4), (4, 5)]      # group slot ranges
            pms = [psM.tile([128, 512], F32, tag="pm", name=f"pm{g}")
                   for g in range(3)]
            for g in range(3):
                nc.tensor.matmul(pms[g][:, 0:GW[g]], w0b, bop(*GS[g]),
                                 start=True, stop=False)
            for g in range(3):
                nc.tensor.matmul(pms[g][:, 0:GW[g]], w0a, nap(*GS[g]),
                                 start=False, stop=True)

            # elu: exp on ACT, fused combine on DVE (GPSIMD can't read PSUM)
            def elu_tile(pv, out_ap, cols, tag):
                """pv: PSUM f32 AP; out_ap: SBUF bf16 dest; cols: elems."""
                e = work.tile([128, cols], BF, tag=tag, name=f"e_{tag}")
                nc.scalar.activation(e[:], pv, AF.Exp)
                nc.vector._custom_dve(ELU_OP, out=out_ap, in0=pv,
                                      in1=e[:], s0=1.0, s1=-1.0)

            m0 = [wp.tile([128, GW[g]], BF, tag=f"m0_{g}", name=f"m0_{g}")
                  for g in range(3)]
            for g in range(3):
                elu_tile(pms[g][:, 0:GW[g]], m0[g][:], GW[g], f"e{g}")

            # ---- msg layer 1 --------------------------------------------
            pm2 = [psN.tile([128, 512], F32, tag="pm2", name=f"pm2_{g}")
                   for g in range(3)]
            for g in range(3):
                nc.tensor.matmul(pm2[g][:, 0:GW[g]], w1, m0[g][:],
                                 start=True, stop=True)
            m1 = [wp.tile([128, GW[g]], BF, tag=f"m1_{g}", name=f"m1_{g}")
                  for g in range(3)]
            for g in range(3):
                elu_tile(pm2[g][:, 0:GW[g]], m1[g][:], GW[g], f"e2{g}")

            # ---- inner layer 0 (degree-5 weights, PSUM accumulate) ------
            pi = pio.tile([128, 512], F32, tag="pio")
            nc.tensor.matmul(pi[:, 0:NA], lo5, nact, start=True, stop=False)
            nc.tensor.matmul(pi[:, 0:NA], hi5, m15, start=False, stop=False)
            for j in range(4):
                nc.tensor.matmul(pi[:, 0:NA], hi5,
                                 m1[j // 2][:, (j % 2) * NA:(j % 2 + 1) * NA],
                                 start=False, stop=False)
            nc.tensor.matmul(pi[:, 0:NA], hi5, m1[2][:], start=False, stop=True)
            h0 = wp.tile([128, NA], BF, tag="h0")
            elu_tile(pi[:, 0:NA], h0[:], NA, "eh")

            # ---- inner layer 1 + output (uneven chunks, two DMA queues) --
            obuf = wp.tile([128, NA], BF, tag="obuf")
            po_a = pio.tile([128, 512], F32, tag="pio")
            nc.tensor.matmul(po_a[:, 0:H], iw15, h0[:, 0:H],
                             start=True, stop=True)
            # reuse a msg0 bank (its readers are long done by now)
            po_b = psM.tile([128, 512], F32, tag="pm", name="po_b")
            nc.tensor.matmul(po_b[:, 0:NA - H], iw15, h0[:, H:NA],
                             start=True, stop=True)
            elu_tile(po_a[:, 0:H], obuf[:, 0:H], H, "eo1")
            nc.sync.dma_start(outp_ap[:, 0:H], obuf[:, 0:H])
            elu_tile(po_b[:, 0:NA - H], obuf[:, H:NA], NA - H, "eo2")
            nc.scalar.dma_start(outp_ap[:, H:NA], obuf[:, H:NA])

    nc.compile()
    return nc


_CACHE = {}


# --------------------------------------------------------------------------
# host side
# --------------------------------------------------------------------------

def _elu(x):
    return np.where(x > 0, x, np.expm1(np.minimum(x, 0.0)))


def _host_fallback(af, bf, ef, deg, ids, msg_w0, msg_w1, inner_w0, inner_w1):
    """Exact f32 reference for the (few) active atoms with degree < 5.
    af: (N,FA) atoms flat; bf: (N,D,FB); ef: (N,D); ids: flat atom indices."""
    if len(ids) == 0:
        return np.zeros((0, CONV), np.float32)
    mol = ids // M
    e = ef[ids]                                   # (n, D)
    nbr = np.where(e[..., None] >= 0,
                   af[(mol[:, None] * M + np.maximum(e, 0)).ravel()]
                   .reshape(len(ids), D, FA),
                   0.0)
    msg_in = np.concatenate([nbr, bf[ids]], axis=-1)        # (n, D, FA+FB)
    msg = _elu(msg_in @ msg_w0)
    msg = _elu(msg @ msg_w1)
    summed = msg.sum(axis=1)                                # (n, MSG)
    s2 = np.concatenate([summed, af[ids]], axis=-1)         # (n, MSG+FA)
    dg = deg[ids]
    h = _elu(np.einsum('nf,nfc->nc', s2, inner_w0[dg]))
    h = _elu(np.einsum('nc,nce->ne', h, inner_w1[dg]))
    return h.astype(np.float32)


def _prep_core(af, bf, ef, ids, NA, msg_w0, msg_w1):
    """Stage one core's deg-5 atoms (flat ids into af/bf/ef).
    Returns (napf [128,5,NA] f32, bopf [32,5,NA] f32, nact [128,NA] f32,
    m15 [128,NA] f32 — the host-computed padding-slot message)."""
    n = len(ids)
    mol = ids // M
    e = ef[ids]                                   # (n, 6), exactly one -1
    real = e >= 0                                 # (n, 6) 5 True per row
    # pack real edges into slots 0-4; the pad slot's bond goes to the host
    order = np.argsort(~real, axis=1, kind="stable")   # real first
    e_p = np.take_along_axis(e, order, axis=1)         # (n,6) col5 = -1
    b_p = np.take_along_axis(bf[ids], order[..., None], axis=1)  # (n,6,FB)

    src = af[(mol[:, None] * M + e_p[:, :5]).ravel()].reshape(n, 5, FA)
    napf = np.zeros((128, 5, NA), np.float32)
    napf[:, :, :n] = src.transpose(2, 1, 0)
    bopf = np.zeros((32, 5, NA), np.float32)
    bopf[:, :, :n] = b_p[:, :5].transpose(2, 1, 0)
    nact = np.zeros((128, NA), np.float32)
    nact[:, :n] = af[ids].T
    # padding-slot bond message: nbr contribution is zero
    m15v = _elu(_elu(b_p[:, 5] @ msg_w0[FA:]) @ msg_w1)   # (n, MSG)
    m15 = np.zeros((128, NA), np.float32)
    m15[:, :n] = m15v.T
    return napf, bopf, nact, m15


def prepare_in_maps(atoms, bonds, edges, msg_w0, msg_w1, inner_w0, inner_w1):
    """Shared by kernel() and test.py: returns (NA, per_core, rest, in_maps)."""
    af = atoms.reshape(B * M, FA)
    bf = bonds.reshape(B * M, D, FB)
    ef = edges.reshape(B * M, D)
    deg = (ef != -1).sum(-1)

    d5 = np.nonzero(deg == 5)[0]
    rest = np.nonzero(deg < 5)[0]

    per_core = [d5[c::NCORES] for c in range(NCORES)]
    NA = max(16, _roundup(max(len(p) for p in per_core), 8))

    awx = np.zeros((128, 256), np.float32)
    awx[:, 0:128] = msg_w0[:FA]
    awx[:, 128:256] = msg_w1
    aw = awx.astype(BF16)
    w0b16 = msg_w0[FA:].astype(BF16)                   # [32,128]
    wbase = np.zeros((128, 384), np.float32)
    wbase[:, 0:128] = inner_w0[5, 128:, :]   # lo5: atom-feature part
    wbase[:, 128:256] = inner_w0[5, :128, :]  # hi5: summed-message part
    wbase[:, 256:384] = inner_w1[5]

    in_maps = []
    for c in range(NCORES):
        ids = per_core[c]
        napf, bopf, nact, m15 = _prep_core(af, bf, ef, ids, NA,
                                           msg_w0, msg_w1)
        napx = napf.reshape(128, 5 * NA).astype(BF16)
        b8 = np.zeros((32, 128 + 5 * NA), BF16)
        b8[:, 0:128] = w0b16
        b8[:, 128:] = bopf.reshape(32, 5 * NA).astype(BF16)
        wbx = np.zeros((128, 384 + 2 * NA), np.float32)
        wbx[:, 0:384] = wbase
        wbx[:, 384:384 + NA] = nact
        wbx[:, 384 + NA:] = m15
        in_maps.append({"aw": aw, "b8": b8, "nap": napx,
                        "wb": wbx.astype(BF16)})
    return NA, per_core, rest, in_maps


def kernel(atoms, bonds, edges, msg_w0, msg_w1, inner_w0, inner_w1):
    atoms = np.asarray(atoms, np.float32)
    bonds = np.asarray(bonds, np.float32)
    edges = np.asarray(edges, np.int32)
    msg_w0 = np.asarray(msg_w0, np.float32)
    msg_w1 = np.asarray(msg_w1, np.float32)
    inner_w0 = np.asarray(inner_w0, np.float32)
    inner_w1 = np.asarray(inner_w1, np.float32)

    NA, per_core, rest, in_maps = prepare_in_maps(
        atoms, bonds, edges, msg_w0, msg_w1, inner_w0, inner_w1)

    if NA not in _CACHE:
        _CACHE[NA] = build_program(NA)
    nc = _CACHE[NA]

    res = bass_utils.run_bass_kernel_spmd(
        nc, in_maps, core_ids=list(range(NCORES)))

    af = atoms.reshape(B * M, FA)
    bf = bonds.reshape(B * M, D, FB)
    ef = edges.reshape(B * M, D)
    deg = (ef != -1).sum(-1)

    out = np.zeros((B * M, CONV), np.float32)
    for c in range(NCORES):
        ids = per_core[c]
        o = np.asarray(res.results[c]["outp"]).astype(np.float32)  # (128, NA)
        out[ids] = o[:, :len(ids)].T
    out[rest] = _host_fallback(af, bf, ef, deg, rest,
                               msg_w0, msg_w1, inner_w0, inner_w1)
    return out.reshape(B, M, CONV)
